# revision 1
# baseline (speedup 1.0000x reference)
"""GQA attention (SEQ=2048, DIM=4096, 32 Q heads / 8 KV heads, head_dim=128),
tensor-parallel over heads across 8 NeuronCores.

Each core owns 4 Q heads + 1 KV head: wq/wk/wv split column-wise, wo split
row-wise; each core produces a partial (2048, 4096) output that the host sums
(the all-reduce of row-parallel wo).

Per-core kernel (matmuls on the float32r PE path: full fp32 operand bytes,
tf32-like rounding, 1 cyc/row at free-dim 512 vs 4 cyc/row for plain fp32):
  A) QKV projections: stream xT (dim-major) blocks; Q^T/K^T/V^T accumulate in
     PSUM over the 4096 contraction; RoPE applied on PSUM eviction; V^T
     transposed back to V via PE transposes.
  B) Flash-style attention per (head, 512-query block): S^T = K^T_blk.T @ Q^T
     (keys on partitions), causal staircase mask added on diagonal blocks,
     exp on ACT (scale=1/sqrt(128) folded in), D = ones.T @ expS^T summed over
     key blocks on the PE, O^T = V_blk.T @ expS^T accumulated in PSUM,
     normalized by 1/D (PE broadcast of the reciprocal) on eviction.
  C) out = O^T.T @ wo accumulated over the 4 heads, streamed to DRAM.
"""

import numpy as np

import concourse.bacc as bacc
import concourse.tile as tile
from concourse import mybir
from concourse.bass_utils import run_bass_kernel_spmd

F32 = mybir.dt.float32
F32R = mybir.dt.float32r

DIM = 4096
SEQ = 2048
HEAD_DIM = 128
N_CORES = 8
QH = 4            # q heads per core
QS = QH * HEAD_DIM  # 512: wq column slice per core
NKT = DIM // 128    # 32 contraction tiles
NSB = SEQ // 512    # 4 sequence blocks
SCALE = 1.0 / float(np.sqrt(HEAD_DIM))
NEG = -1e9


def build_nc():
    nc = bacc.Bacc(trn_type="TRN2")

    xT = nc.declare_dram_parameter("xT", [DIM, SEQ], F32R, isOutput=False)
    wq = nc.declare_dram_parameter("wq", [DIM, QS], F32R, isOutput=False)
    wk = nc.declare_dram_parameter("wk", [DIM, HEAD_DIM], F32R, isOutput=False)
    wv = nc.declare_dram_parameter("wv", [DIM, HEAD_DIM], F32R, isOutput=False)
    wo = nc.declare_dram_parameter("wo", [QS, DIM], F32R, isOutput=False)
    cosT = nc.declare_dram_parameter("cosT", [HEAD_DIM, SEQ], F32, isOutput=False)
    sinTs = nc.declare_dram_parameter("sinTs", [HEAD_DIM, SEQ], F32, isOutput=False)
    stair = nc.declare_dram_parameter("stair", [128, 896], F32, isOutput=False)
    ident = nc.declare_dram_parameter("ident", [128, 128], F32R, isOutput=False)
    ones_col = nc.declare_dram_parameter("ones_col", [128, 1], F32R, isOutput=False)
    ones_row = nc.declare_dram_parameter("ones_row", [1, 128], F32R, isOutput=False)
    out = nc.declare_dram_parameter("out", [SEQ, DIM], F32, isOutput=True)

    with tile.TileContext(nc) as tc:
        with (
            tc.tile_pool(name="persist", bufs=1) as persist,
            tc.tile_pool(name="resid", bufs=1) as resid,
        ):
            # small constants
            stair_sb = persist.tile([128, 896], F32)
            nc.sync.dma_start(out=stair_sb, in_=stair[:, :])
            ident_sb = persist.tile([128, 128], F32R)
            nc.sync.dma_start(out=ident_sb, in_=ident[:, :])
            onesc_sb = persist.tile([128, 1], F32R)
            nc.sync.dma_start(out=onesc_sb, in_=ones_col[:, :])
            onesr_sb = persist.tile([1, 128], F32R)
            nc.sync.dma_start(out=onesr_sb, in_=ones_row[:, :])

            # resident activations
            qT = resid.tile([128, QH, SEQ], F32R)      # Q^T per head (d, seq)
            kT = resid.tile([128, SEQ], F32R)          # K^T (d, seq)
            vN = resid.tile([128, SEQ // 128, 128], F32R)  # V natural (keys, d)

            # ---------------- Phase A: projections + RoPE ----------------
            with (
                tc.tile_pool(name="wpool", bufs=1) as wpool,
                tc.tile_pool(name="xpool", bufs=2) as xpool,
                tc.tile_pool(name="cspool", bufs=2) as cspool,
                tc.tile_pool(name="ropetmp", bufs=2) as ropetmp,
                tc.tile_pool(name="vtb", bufs=2) as vtb,
                tc.tile_pool(name="psA", bufs=1, space="PSUM") as psA,
                tc.tile_pool(name="psVT", bufs=2, space="PSUM") as psVT,
            ):
                # resident weights: per-4kt chunk tiles so the first matmuls
                # only wait on their own 1 MiB DMA, not the whole weight load
                wq_r = wq.rearrange("(t p) m -> p t m", p=128)
                wk_r = wk.rearrange("(t p) m -> p t m", p=128)
                wv_r = wv.rearrange("(t p) m -> p t m", p=128)
                wq_cs, wk_cs, wv_cs = [], [], []
                for c in range(8):
                    wq_cs.append(wpool.tile([128, 4, QS], F32R, name=f"wqc{c}"))
                    wk_cs.append(wpool.tile([128, 4, HEAD_DIM], F32R, name=f"wkc{c}"))
                    wv_cs.append(wpool.tile([128, 4, HEAD_DIM], F32R, name=f"wvc{c}"))

                xT_r = xT.rearrange("(t p) s -> p t s", p=128)

                for sb in range(NSB):
                    ss = slice(sb * 512, (sb + 1) * 512)
                    # PSUM accumulators for this seq block
                    q_ps = [psA.tile([128, 512], F32, tag=f"qps{h}", name=f"qps{h}")
                            for h in range(QH)]
                    k_ps = psA.tile([128, 512], F32, tag="kps")
                    v_ps = psA.tile([128, 512], F32, tag="vps")

                    for g in range(8):  # super-tiles of 4 k-tiles (1 MiB DMAs)
                        if sb == 0:
                            # interleave weight chunks with the x stream so the
                            # first matmuls only queue behind ~2.5 MB of DMA
                            nc.sync.dma_start(
                                out=wq_cs[g], in_=wq_r[:, g * 4:(g + 1) * 4, :]
                            )
                            nc.sync.dma_start(
                                out=wk_cs[g], in_=wk_r[:, g * 4:(g + 1) * 4, :]
                            )
                            nc.sync.dma_start(
                                out=wv_cs[g], in_=wv_r[:, g * 4:(g + 1) * 4, :]
                            )
                        xt = xpool.tile([128, 4, 512], F32R, tag="xt")
                        nc.sync.dma_start(
                            out=xt, in_=xT_r[:, g * 4:(g + 1) * 4, ss]
                        )
                        for i in range(4):
                            kt = g * 4 + i
                            st = (kt == 0)
                            sp = (kt == NKT - 1)
                            for h in range(QH):
                                nc.tensor.matmul(
                                    q_ps[h],
                                    wq_cs[g][:, i, h * 128:(h + 1) * 128],
                                    xt[:, i, :],
                                    start=st, stop=sp,
                                )
                            nc.tensor.matmul(
                                k_ps, wk_cs[g][:, i, :], xt[:, i, :],
                                start=st, stop=sp,
                            )
                            nc.tensor.matmul(
                                v_ps, wv_cs[g][:, i, :], xt[:, i, :],
                                start=st, stop=sp,
                            )

                    # RoPE tables for this block
                    cos_t = cspool.tile([128, 512], F32, tag="cos")
                    nc.sync.dma_start(out=cos_t, in_=cosT[:, ss])
                    sin_t = cspool.tile([128, 512], F32, tag="sin")
                    nc.sync.dma_start(out=sin_t, in_=sinTs[:, ss])

                    def rope(dst, src_ps):
                        # ACT copies release the PSUM bank fast (the next seq
                        # block's matmuls wait on it) and produce both the
                        # straight and half-rotated views, so every DVE op is
                        # partition-aligned (cross-partition SBUF operands are
                        # rejected by the BIR verifier; PSUM->SBUF offset
                        # copies are fine)
                        v = ropetmp.tile([128, 512], F32, tag="v", name="v")
                        vr = ropetmp.tile([128, 512], F32, tag="vr", name="vr")
                        nc.scalar.copy(v, src_ps)
                        nc.scalar.copy(vr[0:64, :], src_ps[64:128, :])
                        nc.scalar.copy(vr[64:128, :], src_ps[0:64, :])
                        t = ropetmp.tile([128, 512], F32, tag="t", name="t")
                        u = ropetmp.tile([128, 512], F32, tag="u", name="u")
                        nc.vector.tensor_mul(t, v, cos_t)
                        nc.vector.tensor_mul(u, vr, sin_t)
                        nc.vector.tensor_add(dst, t, u)

                    # V^T -> V via PE transposes (ACT evicts, keeping DVE free
                    # for RoPE; issued first so the PE transposes overlap ropes)
                    vt_sb = vtb.tile([128, 512], F32R, tag="vt")
                    nc.scalar.copy(vt_sb, v_ps)
                    for j in range(4):
                        vt_ps = psVT.tile([128, 128], F32R, tag="vtp", name="vtp")
                        nc.tensor.transpose(
                            vt_ps, vt_sb[:, j * 128:(j + 1) * 128], ident_sb
                        )
                        nc.scalar.copy(vN[:, sb * 4 + j, :], vt_ps)

                    for h in range(QH):
                        rope(qT[:, h, ss], q_ps[h])
                    rope(kT[:, ss], k_ps)

            # ---------------- Phase B/C: attention + out projection ----------------
            with (
                tc.tile_pool(name="wopool", bufs=1) as wopool,
                tc.tile_pool(name="expp", bufs=8) as expp,
                tc.tile_pool(name="otp", bufs=2) as otp,
                tc.tile_pool(name="dsmall", bufs=2) as dsmall,
                tc.tile_pool(name="bcp", bufs=2) as bcp,
                tc.tile_pool(name="outev", bufs=3) as outev,
                tc.tile_pool(name="psS", bufs=2, space="PSUM") as psS,
                tc.tile_pool(name="psD", bufs=2, space="PSUM") as psD,
                tc.tile_pool(name="psOT", bufs=2, space="PSUM") as psOT,
                tc.tile_pool(name="psC", bufs=2, space="PSUM") as psC,
            ):
                wo_sb = wopool.tile([128, QH, DIM], F32R)
                wo_r = wo.rearrange("(h p) n -> p h n", p=128)
                for h in range(QH):
                    for c in range(2):
                        nc.sync.dma_start(
                            out=wo_sb[:, h, c * 2048:(c + 1) * 2048],
                            in_=wo_r[:, h, c * 2048:(c + 1) * 2048],
                        )

                LAG = 4  # D/AV matmuls trail the score stream by LAG blocks
                for qb in range(NSB):
                    qs = slice(qb * 512, (qb + 1) * 512)
                    n_kb = 4 * qb + 4
                    ot_sb = [None] * QH
                    for h in range(QH):
                        d_ps = psD.tile([1, 512], F32, tag="dps", name="dps")
                        ot_ps = psOT.tile([128, 512], F32, tag="otps", name="otps")
                        ess = [None] * n_kb

                        def drain(kb):
                            nc.tensor.matmul(
                                d_ps, onesc_sb, ess[kb],
                                start=(kb == 0), stop=(kb == n_kb - 1),
                            )
                            nc.tensor.matmul(
                                ot_ps, vN[:, kb, :], ess[kb],
                                start=(kb == 0), stop=(kb == n_kb - 1),
                            )

                        for kb in range(n_kb):
                            s_ps = psS.tile([128, 512], F32, tag="sps", name="sps")
                            nc.tensor.matmul(
                                s_ps,
                                kT[:, kb * 128:(kb + 1) * 128],
                                qT[:, h, qs],
                                start=True, stop=True,
                            )
                            j = kb - 4 * qb
                            if j >= 0:  # diagonal block: causal mask
                                nc.vector.tensor_add(
                                    s_ps, s_ps,
                                    stair_sb[:, 384 - 128 * j:896 - 128 * j],
                                )
                            es = expp.tile([128, 512], F32R, tag="es", name="es")
                            nc.scalar.activation(
                                es, s_ps, mybir.ActivationFunctionType.Exp,
                                scale=SCALE,
                            )
                            ess[kb] = es
                            if kb >= LAG:
                                drain(kb - LAG)
                        for kb in range(max(0, n_kb - LAG), n_kb):
                            drain(kb)
                        # normalize: O^T * (1/D) broadcast across partitions
                        rd = dsmall.tile([1, 512], F32R, tag="rd", name="rd")
                        with nc.allow_low_precision("f32r reciprocal for PE bcast"):
                            nc.vector.reciprocal(rd, d_ps)
                        bc_ps = psS.tile([128, 512], F32, tag="sps", name="bc")
                        nc.tensor.matmul(
                            bc_ps, onesr_sb, rd, start=True, stop=True
                        )
                        bc_sb = bcp.tile([128, 512], F32, tag="bcsb", name="bcsb")
                        nc.scalar.copy(bc_sb, bc_ps)
                        ot = otp.tile([128, 512], F32R, tag=f"ot{h}", name=f"ot{h}")
                        nc.vector.tensor_mul(ot, ot_ps, bc_sb)
                        ot_sb[h] = ot

                    # Phase C for this query block
                    for qc in range(4):
                        for nb in range(8):
                            o_ps = psC.tile([128, 512], F32, tag="ops", name="ops")
                            for h in range(QH):
                                nc.tensor.matmul(
                                    o_ps,
                                    ot_sb[h][:, qc * 128:(qc + 1) * 128],
                                    wo_sb[:, h, nb * 512:(nb + 1) * 512],
                                    start=(h == 0), stop=(h == QH - 1),
                                )
                            ob = outev.tile([128, 512], F32, tag="ob", name="ob")
                            nc.vector.tensor_copy(ob, o_ps)
                            nc.sync.dma_start(
                                out=out[qb * 512 + qc * 128:
                                        qb * 512 + (qc + 1) * 128,
                                        nb * 512:(nb + 1) * 512],
                                in_=ob,
                            )
    nc.finalize()
    return nc


_NC_CACHE = {}


def _get_nc():
    if "nc" not in _NC_CACHE:
        _NC_CACHE["nc"] = build_nc()
    return _NC_CACHE["nc"]


def _host_prep(x, cos, sin, mask, wq, wk, wv, wo):
    xT = np.ascontiguousarray(x[0].T.astype(np.float32))
    cosT = np.ascontiguousarray(cos[:, 0, :].T.astype(np.float32))
    sinT = sin[:, 0, :].T.astype(np.float32)
    sinTs = np.ascontiguousarray(
        np.concatenate([-sinT[:64], sinT[64:]], axis=0)
    )
    rr = np.arange(128, dtype=np.int64)[:, None]
    cc = np.arange(896, dtype=np.int64)[None, :]
    stair = np.where(rr <= cc - 384, 0.0, NEG).astype(np.float32)
    ident = np.eye(128, dtype=np.float32)
    ones_col = np.ones((128, 1), dtype=np.float32)
    ones_row = np.ones((1, 128), dtype=np.float32)

    in_maps = []
    for i in range(N_CORES):
        in_maps.append({
            "xT": xT,
            "wq": np.ascontiguousarray(wq[:, i * QS:(i + 1) * QS]),
            "wk": np.ascontiguousarray(wk[:, i * 128:(i + 1) * 128]),
            "wv": np.ascontiguousarray(wv[:, i * 128:(i + 1) * 128]),
            "wo": np.ascontiguousarray(wo[i * QS:(i + 1) * QS, :]),
            "cosT": cosT,
            "sinTs": sinTs,
            "stair": stair,
            "ident": ident,
            "ones_col": ones_col,
            "ones_row": ones_row,
        })
    return in_maps


def kernel(x, cos, sin, mask, wq, wk, wv, wo, _trace=False, _trace_kwargs=None):
    nc = _get_nc()
    in_maps = _host_prep(x, cos, sin, mask, wq, wk, wv, wo)
    res = run_bass_kernel_spmd(
        nc, in_maps, list(range(N_CORES)), trace=_trace,
        **(_trace_kwargs or {}),
    )
    partials = [res.results[i]["out"] for i in range(N_CORES)]
    full = np.sum(np.stack(partials, axis=0), axis=0, dtype=np.float64)
    out = full.astype(np.float32)[None, :, :]
    if _trace:
        return out, res
    return out



# revision 2
# speedup vs baseline: 1.5116x; 1.5116x over previous
"""GQA attention (SEQ=2048, DIM=4096, 32 Q heads / 8 KV heads, head_dim=128),
tensor-parallel over heads across 8 NeuronCores.

Each core owns 4 Q heads + 1 KV head: wq/wk/wv split column-wise, wo split
row-wise; each core produces a partial (2048, 4096) output that the host sums
(the all-reduce of row-parallel wo).

v2 (vs the f32r baseline at 629us):
 - all matmul operands in bf16 (same PE rate as f32r, but FWL weight loads,
   half the DMA traffic and half the SBUF footprint; PSUM accum stays f32)
 - host pre-packs every DRAM tensor so each DMA's per-partition line is
   contiguous (>=4KB descriptors), x/out on the sync HWDGE ring, weights and
   tables on the scalar HWDGE ring
 - phase A runs K/V matmuls of each seq block before the Q matmuls and
   double-buffers the K/V PSUM banks, so the next block's PE work never waits
   on the PSUM eviction (RoPE) of the previous one; V^T->V transposes moved
   off the PE onto the DMA xbar (dma_start_transpose)
 - softmax denominator accumulated as broadcast rows via an all-ones lhsT
   (same PE cost, no separate 1->128 broadcast matmul) and inverted with
   reciprocal_approx_fast (~5x faster than reciprocal, which was a 3.3us
   critical-path stall per head)
 - phase C (out-projection) for query block qb is issued interleaved into the
   attention streams of qb+1, so the PE never idles on the normalization
   chain; PSUM evictions alternate between ACT and DVE
"""

import numpy as np
import ml_dtypes

import concourse.bacc as bacc
import concourse.tile as tile
from concourse import mybir
from concourse.bass_utils import run_bass_kernel_spmd

F32 = mybir.dt.float32
BF16 = mybir.dt.bfloat16
BF_NP = ml_dtypes.bfloat16

DIM = 4096
SEQ = 2048
HEAD_DIM = 128
N_CORES = 8
QH = 4              # q heads per core
QS = QH * HEAD_DIM  # 512: wq column slice per core
NKT = DIM // 128    # 32 contraction tiles
NSB = SEQ // 512    # 4 sequence blocks
NCH = 8             # x/weight super-chunks per seq block (4 k-tiles each)
SCALE = 1.0 / float(np.sqrt(HEAD_DIM))
NEG = -1e9
LAG = 5             # D/AV matmuls trail the score stream by LAG blocks


def build_nc():
    nc = bacc.Bacc(trn_type="TRN2")

    xP = nc.declare_dram_parameter("xP", [NSB * NCH * 128, 2048], BF16, isOutput=False)
    wqP = nc.declare_dram_parameter("wqP", [NCH * 128, 2048], BF16, isOutput=False)
    wkP = nc.declare_dram_parameter("wkP", [NCH * 128, 512], BF16, isOutput=False)
    wvP = nc.declare_dram_parameter("wvP", [NCH * 128, 512], BF16, isOutput=False)
    woP = nc.declare_dram_parameter("woP", [128, QH * DIM], BF16, isOutput=False)
    cosP = nc.declare_dram_parameter("cosP", [128, SEQ], F32, isOutput=False)
    sinP = nc.declare_dram_parameter("sinP", [128, SEQ], F32, isOutput=False)
    stairP = nc.declare_dram_parameter("stairP", [128, 896], F32, isOutput=False)
    onesP = nc.declare_dram_parameter("onesP", [128, 128], BF16, isOutput=False)
    out = nc.declare_dram_parameter("out", [SEQ, DIM], BF16, isOutput=True)

    xr = xP.rearrange("(c p) s -> p c s", p=128)
    wqr = wqP.rearrange("(c p) m -> p c m", p=128)
    wkr = wkP.rearrange("(c p) m -> p c m", p=128)
    wvr = wvP.rearrange("(c p) m -> p c m", p=128)

    with tile.TileContext(nc) as tc:
        with (
            tc.tile_pool(name="persist", bufs=1) as persist,
            tc.tile_pool(name="resid", bufs=1) as resid,
        ):
            # small constants + wo, on the scalar/sync rings before the x
            # stream floods them
            stair_sb = persist.tile([128, 896], F32)
            nc.sync.dma_start(out=stair_sb, in_=stairP[:, :])
            ones_sb = persist.tile([128, 128], BF16)
            nc.sync.dma_start(out=ones_sb, in_=onesP[:, :])
            wo_sb = persist.tile([128, QH * DIM], BF16)

            # resident activations (all bf16)
            qT = resid.tile([128, QH, SEQ], BF16)      # Q^T per head (d, seq)
            kT = resid.tile([128, SEQ], BF16)          # K^T (d, seq)
            vN = resid.tile([128, SEQ // 128, 128], BF16)  # V natural (keys, d)

            # ---------------- Phase A: projections + RoPE ----------------
            with (
                tc.tile_pool(name="wpool", bufs=1) as wpool,
                tc.tile_pool(name="xpool", bufs=1) as xpool,
                tc.tile_pool(name="cspool", bufs=1) as cspool,
                tc.tile_pool(name="ropetmp", bufs=2) as ropetmp,
                tc.tile_pool(name="vtb", bufs=2) as vtb,
                tc.tile_pool(name="psQ", bufs=1, space="PSUM") as psQ,
                tc.tile_pool(name="psKV", bufs=2, space="PSUM") as psKV,
            ):
                cos_sb = cspool.tile([128, SEQ], F32)
                sin_sb = cspool.tile([128, SEQ], F32)

                wq_cs, wk_cs, wv_cs = [], [], []
                for g in range(NCH):
                    wq_cs.append(wpool.tile([128, 2048], BF16, name=f"wqc{g}"))
                    wk_cs.append(wpool.tile([128, 512], BF16, name=f"wkc{g}"))
                    wv_cs.append(wpool.tile([128, 512], BF16, name=f"wvc{g}"))

                xts = [None] * NCH

                def rope(dst, src_ps, ss):
                    # rotate-half via two partition-offset PSUM->SBUF copies
                    # on ACT; multiplies/add on DVE (sin sign pre-folded)
                    vr = ropetmp.tile([128, 512], F32, tag="vr", name="vr")
                    nc.scalar.copy(vr[0:64, :], src_ps[64:128, :])
                    nc.scalar.copy(vr[64:128, :], src_ps[0:64, :])
                    t = ropetmp.tile([128, 512], F32, tag="t", name="t")
                    u = ropetmp.tile([128, 512], F32, tag="u", name="u")
                    nc.vector.tensor_mul(t, src_ps, cos_sb[:, ss])
                    nc.vector.tensor_mul(u, vr, sin_sb[:, ss])
                    nc.vector.tensor_add(dst, t, u)

                for sb in range(NSB):
                    ss = slice(sb * 512, (sb + 1) * 512)
                    q_ps = [psQ.tile([128, 512], F32, tag=f"qps{h}", name=f"qps{h}")
                            for h in range(QH)]
                    k_ps = psKV.tile([128, 512], F32, tag="kps", name="kps")
                    v_ps = psKV.tile([128, 512], F32, tag="vps", name="vps")

                    # DMAs for this seq block: x chunks on the sync ring;
                    # (first block only) weights on the scalar ring, then
                    # RoPE tables, then wo
                    for g in range(NCH):
                        if sb == 0:
                            nc.scalar.dma_start(out=wq_cs[g], in_=wqr[:, g, :])
                            nc.scalar.dma_start(out=wk_cs[g], in_=wkr[:, g, :])
                            nc.scalar.dma_start(out=wv_cs[g], in_=wvr[:, g, :])
                        xt = xpool.tile([128, 2048], BF16, tag=f"x{g}",
                                        name=f"x{g}")
                        nc.sync.dma_start(out=xt, in_=xr[:, sb * NCH + g, :])
                        xts[g] = xt
                    if sb == 0:
                        nc.scalar.dma_start(out=cos_sb, in_=cosP[:, :])
                        nc.scalar.dma_start(out=sin_sb, in_=sinP[:, :])
                        for c in range(4):
                            cs = slice(c * QH * DIM // 4, (c + 1) * QH * DIM // 4)
                            nc.scalar.dma_start(out=wo_sb[:, cs], in_=woP[:, cs])

                    def kv_pass(gs):
                        for g in gs:
                            for i in range(4):
                                kt_i = g * 4 + i
                                st = (kt_i == 0)
                                sp = (kt_i == NKT - 1)
                                xsl = xts[g][:, i * 512:(i + 1) * 512]
                                nc.tensor.matmul(
                                    k_ps, wk_cs[g][:, i * 128:(i + 1) * 128],
                                    xsl, start=st, stop=sp,
                                )
                                nc.tensor.matmul(
                                    v_ps, wv_cs[g][:, i * 128:(i + 1) * 128],
                                    xsl, start=st, stop=sp,
                                )

                    def q_pass(gs):
                        for g in gs:
                            for i in range(4):
                                kt_i = g * 4 + i
                                st = (kt_i == 0)
                                sp = (kt_i == NKT - 1)
                                xsl = xts[g][:, i * 512:(i + 1) * 512]
                                for h in range(QH):
                                    nc.tensor.matmul(
                                        q_ps[h],
                                        wq_cs[g][:, i * 512 + h * 128:
                                                 i * 512 + (h + 1) * 128],
                                        xsl, start=st, stop=sp,
                                    )

                    # K/V of the next block never waits on Q evictions, and
                    # Q evictions overlap the second-half Q matmuls
                    kv_pass(range(0, 4))
                    q_pass(range(0, 4))
                    kv_pass(range(4, 8))

                    # K/V eviction (K rope + V transpose via DMA xbar),
                    # overlapped with the second-half Q matmuls below
                    rope(kT[:, ss], k_ps, ss)
                    vt_sb = vtb.tile([128, 512], BF16, tag="vt", name="vt")
                    nc.scalar.copy(vt_sb, v_ps)
                    for j in range(4):
                        nc.sync.dma_start_transpose(
                            out=vN[:, sb * 4 + j, :],
                            in_=vt_sb[:, j * 128:(j + 1) * 128],
                        )

                    q_pass(range(4, 8))

                    for h in range(QH):
                        rope(qT[:, h, ss], q_ps[h], ss)

            # ---------------- Phase B/C: attention + out projection ----------------
            with (
                tc.tile_pool(name="expp", bufs=10) as expp,
                tc.tile_pool(name="otp", bufs=2) as otp,
                tc.tile_pool(name="rdp", bufs=2) as rdp,
                tc.tile_pool(name="outev", bufs=3) as outev,
                tc.tile_pool(name="psS", bufs=2, space="PSUM") as psS,
                tc.tile_pool(name="psD", bufs=2, space="PSUM") as psD,
                tc.tile_pool(name="psOT", bufs=2, space="PSUM") as psOT,
                tc.tile_pool(name="psC", bufs=2, space="PSUM") as psC,
            ):
                ot_hist = [[None] * QH for _ in range(NSB)]

                def c_chunk(qbc, qc):
                    # out rows [qbc*512 + qc*128, +128) x all 4096 cols
                    ob = outev.tile([128, DIM], BF16, tag="ob", name="ob")
                    for nb in range(8):
                        o_ps = psC.tile([128, 512], F32, tag="ops", name="ops")
                        for h2 in range(QH):
                            nc.tensor.matmul(
                                o_ps,
                                ot_hist[qbc][h2][:, qc * 128:(qc + 1) * 128],
                                wo_sb[:, h2 * DIM + nb * 512:
                                      h2 * DIM + (nb + 1) * 512],
                                start=(h2 == 0), stop=(h2 == QH - 1),
                            )
                        osl = ob[:, nb * 512:(nb + 1) * 512]
                        if nb % 2 == 0:
                            nc.scalar.copy(osl, o_ps)
                        else:
                            nc.vector.tensor_copy(osl, o_ps)
                    nc.sync.dma_start(
                        out=out[qbc * 512 + qc * 128:
                                qbc * 512 + (qc + 1) * 128, :],
                        in_=ob,
                    )

                for qb in range(NSB):
                    qs = slice(qb * 512, (qb + 1) * 512)
                    n_kb = 4 * qb + 4
                    for h in range(QH):
                        d_ps = psD.tile([128, 512], F32, tag="dps", name="dps")
                        ot_ps = psOT.tile([128, 512], F32, tag="otps",
                                          name="otps")
                        ess = [None] * n_kb

                        def drain(kb):
                            nc.tensor.matmul(
                                d_ps, ones_sb, ess[kb],
                                start=(kb == 0), stop=(kb == n_kb - 1),
                            )
                            nc.tensor.matmul(
                                ot_ps, vN[:, kb, :], ess[kb],
                                start=(kb == 0), stop=(kb == n_kb - 1),
                            )

                        for kb in range(n_kb):
                            s_ps = psS.tile([128, 512], F32, tag="sps",
                                            name="sps")
                            nc.tensor.matmul(
                                s_ps,
                                kT[:, kb * 128:(kb + 1) * 128],
                                qT[:, h, qs],
                                start=True, stop=True,
                            )
                            j = kb - 4 * qb
                            if j >= 0:  # diagonal block: causal staircase
                                nc.vector.tensor_add(
                                    s_ps, s_ps,
                                    stair_sb[:, 384 - 128 * j:896 - 128 * j],
                                )
                            es = expp.tile([128, 512], BF16, tag="es",
                                           name="es")
                            nc.scalar.activation(
                                es, s_ps, mybir.ActivationFunctionType.Exp,
                                scale=SCALE,
                            )
                            ess[kb] = es
                            if kb >= LAG:
                                drain(kb - LAG)
                        for kb in range(max(0, n_kb - LAG), n_kb):
                            drain(kb)

                        # normalization: D rows are already broadcast across
                        # all 128 partitions (all-ones lhsT), so 1/D is a
                        # straight elementwise op feeding the O^T scaling
                        rd = rdp.tile([128, 512], F32, tag="rd", name="rd")
                        nc.vector.reciprocal_approx_fast(rd, d_ps)
                        ot = otp.tile([128, 512], BF16, tag=f"ot{h}",
                                      name=f"ot{h}")
                        nc.vector.tensor_mul(ot, ot_ps, rd)
                        ot_hist[qb][h] = ot

                        # keep the PE fed while the chain above retires:
                        # one quarter of the previous block's out-projection
                        if qb >= 1:
                            c_chunk(qb - 1, h)

                for qc in range(4):
                    c_chunk(NSB - 1, qc)

    nc.finalize()
    return nc


_NC_CACHE = {}


def _get_nc():
    if "nc" not in _NC_CACHE:
        _NC_CACHE["nc"] = build_nc()
    return _NC_CACHE["nc"]


def _host_prep(x, cos, sin, mask, wq, wk, wv, wo):
    xT = np.ascontiguousarray(x[0].T.astype(np.float32))  # [DIM, SEQ]
    # x chunk (sb, g) holds k-tiles 4g..4g+3, seq cols [512sb, 512sb+512):
    # layout [sb, g, p, i, s'] so each DMA partition line is 4KB contiguous
    x5 = xT.reshape(NCH, 4, 128, NSB, 512)        # [g, i, p, sb, s']
    xPf = np.transpose(x5, (3, 0, 2, 1, 4))       # [sb, g, p, i, s']
    xP = np.ascontiguousarray(
        xPf.astype(BF_NP).reshape(NSB * NCH * 128, 2048))

    cosT = np.ascontiguousarray(cos[:, 0, :].T.astype(np.float32))
    sinT = sin[:, 0, :].T.astype(np.float32)
    sinTs = np.ascontiguousarray(
        np.concatenate([-sinT[:64], sinT[64:]], axis=0))

    rr = np.arange(128, dtype=np.int64)[:, None]
    cc = np.arange(896, dtype=np.int64)[None, :]
    stair = np.where(rr <= cc - 384, 0.0, NEG).astype(np.float32)
    ones = np.ones((128, 128), dtype=BF_NP)

    def pack_w(w_slice, m):
        # [DIM, m] -> [g, p, i, m] with per-partition contiguous (i, m)
        w4 = w_slice.reshape(NCH, 4, 128, m)       # [g, i, p, m]
        wf = np.transpose(w4, (0, 2, 1, 3))        # [g, p, i, m]
        return np.ascontiguousarray(
            wf.astype(BF_NP).reshape(NCH * 128, 4 * m))

    in_maps = []
    for i in range(N_CORES):
        wo_c = wo[i * QS:(i + 1) * QS, :]          # [512, DIM]
        wo4 = wo_c.reshape(QH, 128, DIM)           # [h, p, n]
        woPf = np.ascontiguousarray(
            np.transpose(wo4, (1, 0, 2)).astype(BF_NP).reshape(128, QH * DIM))
        in_maps.append({
            "xP": xP,
            "wqP": pack_w(wq[:, i * QS:(i + 1) * QS], 512),
            "wkP": pack_w(wk[:, i * 128:(i + 1) * 128], 128),
            "wvP": pack_w(wv[:, i * 128:(i + 1) * 128], 128),
            "woP": woPf,
            "cosP": cosT,
            "sinP": sinTs,
            "stairP": stair,
            "onesP": ones,
        })
    return in_maps


def kernel(x, cos, sin, mask, wq, wk, wv, wo, _trace=False, _trace_kwargs=None):
    nc = _get_nc()
    in_maps = _host_prep(x, cos, sin, mask, wq, wk, wv, wo)
    res = run_bass_kernel_spmd(
        nc, in_maps, list(range(N_CORES)), trace=_trace,
        **(_trace_kwargs or {}),
    )
    partials = [np.asarray(res.results[i]["out"], dtype=np.float32)
                for i in range(N_CORES)]
    full = np.sum(np.stack(partials, axis=0), axis=0, dtype=np.float64)
    out = full.astype(np.float32)[None, :, :]
    if _trace:
        return out, res
    return out


# revision 6
# speedup vs baseline: 1.5811x; 1.0460x over previous
"""GQA attention (SEQ=2048, DIM=4096, 32 Q heads / 8 KV heads, head_dim=128),
tensor-parallel over heads across 8 NeuronCores.

Each core owns 4 Q heads + 1 KV head: wq/wk/wv split column-wise, wo split
row-wise; each core produces a partial (2048, 4096) output that the host sums
(the all-reduce of row-parallel wo).

v2 (vs the f32r baseline at 629us):
 - all matmul operands in bf16 (same PE rate as f32r, but FWL weight loads,
   half the DMA traffic and half the SBUF footprint; PSUM accum stays f32)
 - host pre-packs every DRAM tensor so each DMA's per-partition line is
   contiguous (>=4KB descriptors), x/out on the sync HWDGE ring, weights and
   tables on the scalar HWDGE ring
 - phase A runs K/V matmuls of each seq block before the Q matmuls and
   double-buffers the K/V PSUM banks, so the next block's PE work never waits
   on the PSUM eviction (RoPE) of the previous one; V^T->V transposes moved
   off the PE onto the DMA xbar (dma_start_transpose)
 - softmax denominator accumulated as broadcast rows via an all-ones lhsT
   (same PE cost, no separate 1->128 broadcast matmul) and inverted with
   reciprocal_approx_fast (~5x faster than reciprocal, which was a 3.3us
   critical-path stall per head)
 - phase C (out-projection) for query block qb is issued interleaved into the
   attention streams of qb+1, so the PE never idles on the normalization
   chain; PSUM evictions alternate between ACT and DVE
"""

import numpy as np
import ml_dtypes

import concourse.bacc as bacc
import concourse.tile as tile
from concourse import mybir
from concourse.bass_utils import run_bass_kernel_spmd

F32 = mybir.dt.float32
BF16 = mybir.dt.bfloat16
BF_NP = ml_dtypes.bfloat16

DIM = 4096
SEQ = 2048
HEAD_DIM = 128
N_CORES = 8
QH = 4              # q heads per core
QS = QH * HEAD_DIM  # 512: wq column slice per core
NKT = DIM // 128    # 32 contraction tiles
NSB = SEQ // 512    # 4 sequence blocks
NCH = 8             # x/weight super-chunks per seq block (4 k-tiles each)
SCALE = 1.0 / float(np.sqrt(HEAD_DIM))
NEG = -1e9
LAG = 5             # D/AV matmuls trail the score stream by LAG blocks


def build_nc():
    nc = bacc.Bacc(trn_type="TRN2")

    xP = nc.declare_dram_parameter("xP", [NSB * NCH * 128, 2048], BF16, isOutput=False)
    wqP = nc.declare_dram_parameter("wqP", [NCH * 128, 2048], BF16, isOutput=False)
    wkP = nc.declare_dram_parameter("wkP", [NCH * 128, 512], BF16, isOutput=False)
    wvP = nc.declare_dram_parameter("wvP", [NCH * 128, 512], BF16, isOutput=False)
    woP = nc.declare_dram_parameter("woP", [128, QH * DIM], BF16, isOutput=False)
    cosP = nc.declare_dram_parameter("cosP", [128, SEQ], F32, isOutput=False)
    sinP = nc.declare_dram_parameter("sinP", [128, SEQ], F32, isOutput=False)
    stairP = nc.declare_dram_parameter("stairP", [128, 896], F32, isOutput=False)
    onesP = nc.declare_dram_parameter("onesP", [128, 128], BF16, isOutput=False)
    out = nc.declare_dram_parameter("out", [SEQ, DIM], BF16, isOutput=True)

    xr = xP.rearrange("(c p) s -> p c s", p=128)
    wqr = wqP.rearrange("(c p) m -> p c m", p=128)
    wkr = wkP.rearrange("(c p) m -> p c m", p=128)
    wvr = wvP.rearrange("(c p) m -> p c m", p=128)

    with tile.TileContext(nc) as tc:
        with (
            tc.tile_pool(name="persist", bufs=1) as persist,
            tc.tile_pool(name="resid", bufs=1) as resid,
        ):
            # small constants + wo, on the scalar/sync rings before the x
            # stream floods them
            stair_sb = persist.tile([128, 896], F32)
            nc.sync.dma_start(out=stair_sb, in_=stairP[:, :])
            ones_sb = persist.tile([128, 128], BF16)
            nc.sync.dma_start(out=ones_sb, in_=onesP[:, :])
            wo_sb = persist.tile([128, QH * DIM], BF16)

            # resident activations (all bf16)
            qT = resid.tile([128, QH, SEQ], BF16)      # Q^T per head (d, seq)
            kT = resid.tile([128, SEQ], BF16)          # K^T (d, seq)
            vN = resid.tile([128, SEQ // 128, 128], BF16)  # V natural (keys, d)

            # ---------------- Phase A: projections + RoPE ----------------
            with (
                tc.tile_pool(name="wpool", bufs=1) as wpool,
                tc.tile_pool(name="xpool", bufs=1) as xpool,
                tc.tile_pool(name="cspool", bufs=1) as cspool,
                tc.tile_pool(name="ropetmp", bufs=2) as ropetmp,
                tc.tile_pool(name="vtb", bufs=2) as vtb,
                tc.tile_pool(name="psQ", bufs=1, space="PSUM") as psQ,
                tc.tile_pool(name="psKV", bufs=2, space="PSUM") as psKV,
            ):
                cos_sb = cspool.tile([128, SEQ], F32)
                sin_sb = cspool.tile([128, SEQ], F32)

                wq_cs, wk_cs, wv_cs = [], [], []
                for g in range(NCH):
                    wq_cs.append(wpool.tile([128, 2048], BF16, name=f"wqc{g}"))
                    wk_cs.append(wpool.tile([128, 512], BF16, name=f"wkc{g}"))
                    wv_cs.append(wpool.tile([128, 512], BF16, name=f"wvc{g}"))

                xts = [None] * NCH

                def rope(dst, src_ps, ss):
                    # rotate-half via two partition-offset PSUM->SBUF copies
                    # on ACT; multiplies/add on DVE (sin sign pre-folded)
                    vr = ropetmp.tile([128, 512], F32, tag="vr", name="vr")
                    nc.scalar.copy(vr[0:64, :], src_ps[64:128, :])
                    nc.scalar.copy(vr[64:128, :], src_ps[0:64, :])
                    t = ropetmp.tile([128, 512], F32, tag="t", name="t")
                    u = ropetmp.tile([128, 512], F32, tag="u", name="u")
                    nc.vector.tensor_mul(t, src_ps, cos_sb[:, ss])
                    nc.vector.tensor_mul(u, vr, sin_sb[:, ss])
                    nc.vector.tensor_add(dst, t, u)

                for sb in range(NSB):
                    ss = slice(sb * 512, (sb + 1) * 512)
                    q_ps = [psQ.tile([128, 512], F32, tag=f"qps{h}", name=f"qps{h}")
                            for h in range(QH)]
                    k_ps = psKV.tile([128, 512], F32, tag="kps", name="kps")
                    v_ps = psKV.tile([128, 512], F32, tag="vps", name="vps")

                    # DMAs for this seq block: x chunks on the sync ring;
                    # (first block only) weights on the scalar ring in
                    # consumption order (kv pass g0-3 / q pass g0-3 / ...),
                    # then RoPE tables, then wo
                    for g in range(NCH):
                        xt = xpool.tile([128, 2048], BF16, tag=f"x{g}",
                                        name=f"x{g}")
                        nc.sync.dma_start(out=xt, in_=xr[:, sb * NCH + g, :])
                        xts[g] = xt
                    if sb == 0:
                        for gs in (range(0, 4), range(4, 8)):
                            for g in gs:
                                nc.scalar.dma_start(out=wk_cs[g],
                                                    in_=wkr[:, g, :])
                                nc.scalar.dma_start(out=wv_cs[g],
                                                    in_=wvr[:, g, :])
                            for g in gs:
                                nc.scalar.dma_start(out=wq_cs[g],
                                                    in_=wqr[:, g, :])
                        nc.scalar.dma_start(out=cos_sb, in_=cosP[:, :])
                        nc.scalar.dma_start(out=sin_sb, in_=sinP[:, :])
                        for c in range(4):
                            cs = slice(c * QH * DIM // 4, (c + 1) * QH * DIM // 4)
                            nc.scalar.dma_start(out=wo_sb[:, cs], in_=woP[:, cs])

                    def kv_pass(gs):
                        for g in gs:
                            for i in range(4):
                                kt_i = g * 4 + i
                                st = (kt_i == 0)
                                sp = (kt_i == NKT - 1)
                                xsl = xts[g][:, i * 512:(i + 1) * 512]
                                nc.tensor.matmul(
                                    k_ps, wk_cs[g][:, i * 128:(i + 1) * 128],
                                    xsl, start=st, stop=sp,
                                )
                                nc.tensor.matmul(
                                    v_ps, wv_cs[g][:, i * 128:(i + 1) * 128],
                                    xsl, start=st, stop=sp,
                                )

                    def q_pass(gs):
                        for g in gs:
                            for i in range(4):
                                kt_i = g * 4 + i
                                st = (kt_i == 0)
                                sp = (kt_i == NKT - 1)
                                xsl = xts[g][:, i * 512:(i + 1) * 512]
                                for h in range(QH):
                                    nc.tensor.matmul(
                                        q_ps[h],
                                        wq_cs[g][:, i * 512 + h * 128:
                                                 i * 512 + (h + 1) * 128],
                                        xsl, start=st, stop=sp,
                                    )

                    # K/V of the next block never waits on Q evictions, and
                    # Q evictions overlap the second-half Q matmuls
                    kv_pass(range(0, 4))
                    q_pass(range(0, 4))
                    kv_pass(range(4, 8))

                    # K/V eviction (K rope + V transpose via DMA xbar),
                    # overlapped with the second-half Q matmuls below
                    rope(kT[:, ss], k_ps, ss)
                    vt_sb = vtb.tile([128, 512], BF16, tag="vt", name="vt")
                    nc.scalar.copy(vt_sb, v_ps)
                    for j in range(4):
                        nc.sync.dma_start_transpose(
                            out=vN[:, sb * 4 + j, :],
                            in_=vt_sb[:, j * 128:(j + 1) * 128],
                        )

                    q_pass(range(4, 8))

                    # staged Q RoPE: the PSUM-freeing reads (ACT half-copies
                    # + DVE cos-mul) for all four heads go first, so the next
                    # block's (or phase B's) matmuls get their banks back in
                    # ~3us instead of ~8us
                    vrs, ts = [], []
                    for h in range(QH):
                        vr = ropetmp.tile([128, 512], F32, tag=f"qvr{h}",
                                          name=f"qvr{h}")
                        nc.scalar.copy(vr[0:64, :], q_ps[h][64:128, :])
                        nc.scalar.copy(vr[64:128, :], q_ps[h][0:64, :])
                        t = ropetmp.tile([128, 512], F32, tag=f"qt{h}",
                                         name=f"qt{h}")
                        nc.vector.tensor_mul(t, q_ps[h], cos_sb[:, ss])
                        vrs.append(vr)
                        ts.append(t)
                    for h in range(QH):
                        u = ropetmp.tile([128, 512], F32, tag="qu", name="qu")
                        nc.vector.tensor_mul(u, vrs[h], sin_sb[:, ss])
                        nc.vector.tensor_add(qT[:, h, ss], ts[h], u)

            # ---------------- Phase B/C: attention + out projection ----------------
            with (
                tc.tile_pool(name="expp", bufs=10) as expp,
                tc.tile_pool(name="otp", bufs=2) as otp,
                tc.tile_pool(name="rdp", bufs=2) as rdp,
                tc.tile_pool(name="outev", bufs=3) as outev,
                tc.tile_pool(name="psS", bufs=2, space="PSUM") as psS,
                tc.tile_pool(name="psD", bufs=2, space="PSUM") as psD,
                tc.tile_pool(name="psOT", bufs=2, space="PSUM") as psOT,
                tc.tile_pool(name="psC", bufs=2, space="PSUM") as psC,
            ):
                ot_hist = [[None] * QH for _ in range(NSB)]

                def c_chunk(qbc, qc):
                    # out rows [qbc*512 + qc*128, +128) x all 4096 cols
                    ob = outev.tile([128, DIM], BF16, tag="ob", name="ob")
                    for nb in range(8):
                        o_ps = psC.tile([128, 512], F32, tag="ops", name="ops")
                        for h2 in range(QH):
                            nc.tensor.matmul(
                                o_ps,
                                ot_hist[qbc][h2][:, qc * 128:(qc + 1) * 128],
                                wo_sb[:, h2 * DIM + nb * 512:
                                      h2 * DIM + (nb + 1) * 512],
                                start=(h2 == 0), stop=(h2 == QH - 1),
                            )
                        osl = ob[:, nb * 512:(nb + 1) * 512]
                        if nb % 2 == 0:
                            nc.scalar.copy(osl, o_ps)
                        else:
                            nc.vector.tensor_copy(osl, o_ps)
                        if nb in (3, 7):  # drain each half as soon as ready
                            rows = slice(qbc * 512 + qc * 128,
                                         qbc * 512 + (qc + 1) * 128)
                            cols = slice((nb - 3) * 512, (nb + 1) * 512)
                            nc.sync.dma_start(out=out[rows, cols],
                                              in_=ob[:, cols])

                for qb in range(NSB):
                    qs = slice(qb * 512, (qb + 1) * 512)
                    n_kb = 4 * qb + 4
                    for h in range(QH):
                        d_ps = psD.tile([128, 512], F32, tag="dps", name="dps")
                        ot_ps = psOT.tile([128, 512], F32, tag="otps",
                                          name="otps")
                        ess = [None] * n_kb

                        def drain(kb):
                            es, qoff, vw = ess[kb]
                            st = (kb == 0)
                            sp = (kb == n_kb - 1)
                            nc.tensor.matmul(
                                d_ps[:, qoff:512], ones_sb, es[:, 0:vw],
                                start=st, stop=sp,
                            )
                            nc.tensor.matmul(
                                ot_ps[:, qoff:512], vN[:, kb, :], es[:, 0:vw],
                                start=st, stop=sp,
                            )

                        for kb in range(n_kb):
                            # diagonal blocks: queries below the staircase see
                            # no valid key -> stream only the live columns
                            j = kb - 4 * qb
                            qoff = 128 * j if j > 0 else 0
                            vw = 512 - qoff
                            s_ps = psS.tile([128, 512], F32, tag="sps",
                                            name="sps")
                            nc.tensor.matmul(
                                s_ps[:, 0:vw],
                                kT[:, kb * 128:(kb + 1) * 128],
                                qT[:, h, qb * 512 + qoff:(qb + 1) * 512],
                                start=True, stop=True,
                            )
                            if j >= 0:  # causal staircase within the block
                                nc.vector.tensor_add(
                                    s_ps[:, 0:vw], s_ps[:, 0:vw],
                                    stair_sb[:, 384:384 + vw],
                                )
                            es = expp.tile([128, 512], BF16, tag="es",
                                           name="es")
                            nc.scalar.activation(
                                es[:, 0:vw], s_ps[:, 0:vw],
                                mybir.ActivationFunctionType.Exp,
                                scale=SCALE,
                            )
                            ess[kb] = (es, qoff, vw)
                            if kb >= LAG:
                                drain(kb - LAG)
                        for kb in range(max(0, n_kb - LAG), n_kb):
                            drain(kb)

                        # normalization: D rows are already broadcast across
                        # all 128 partitions (all-ones lhsT), so 1/D is a
                        # straight elementwise op feeding the O^T scaling
                        rd = rdp.tile([128, 512], F32, tag="rd", name="rd")
                        nc.vector.reciprocal_approx_fast(rd, d_ps)
                        ot = otp.tile([128, 512], BF16, tag=f"ot{h}",
                                      name=f"ot{h}")
                        nc.vector.tensor_mul(ot, ot_ps, rd)
                        ot_hist[qb][h] = ot

                        # keep the PE fed while the chain above retires:
                        # one quarter of the previous block's out-projection
                        if qb >= 1:
                            c_chunk(qb - 1, h)

                for qc in range(4):
                    c_chunk(NSB - 1, qc)

    nc.finalize()
    return nc


_NC_CACHE = {}


def _get_nc():
    if "nc" not in _NC_CACHE:
        _NC_CACHE["nc"] = build_nc()
    return _NC_CACHE["nc"]


def _host_prep(x, cos, sin, mask, wq, wk, wv, wo):
    xT = np.ascontiguousarray(x[0].T.astype(np.float32))  # [DIM, SEQ]
    # x chunk (sb, g) holds k-tiles 4g..4g+3, seq cols [512sb, 512sb+512):
    # layout [sb, g, p, i, s'] so each DMA partition line is 4KB contiguous
    x5 = xT.reshape(NCH, 4, 128, NSB, 512)        # [g, i, p, sb, s']
    xPf = np.transpose(x5, (3, 0, 2, 1, 4))       # [sb, g, p, i, s']
    xP = np.ascontiguousarray(
        xPf.astype(BF_NP).reshape(NSB * NCH * 128, 2048))

    cosT = np.ascontiguousarray(cos[:, 0, :].T.astype(np.float32))
    sinT = sin[:, 0, :].T.astype(np.float32)
    sinTs = np.ascontiguousarray(
        np.concatenate([-sinT[:64], sinT[64:]], axis=0))

    rr = np.arange(128, dtype=np.int64)[:, None]
    cc = np.arange(896, dtype=np.int64)[None, :]
    stair = np.where(rr <= cc - 384, 0.0, NEG).astype(np.float32)
    ones = np.ones((128, 128), dtype=BF_NP)

    def pack_w(w_slice, m):
        # [DIM, m] -> [g, p, i, m] with per-partition contiguous (i, m)
        w4 = w_slice.reshape(NCH, 4, 128, m)       # [g, i, p, m]
        wf = np.transpose(w4, (0, 2, 1, 3))        # [g, p, i, m]
        return np.ascontiguousarray(
            wf.astype(BF_NP).reshape(NCH * 128, 4 * m))

    in_maps = []
    for i in range(N_CORES):
        wo_c = wo[i * QS:(i + 1) * QS, :]          # [512, DIM]
        wo4 = wo_c.reshape(QH, 128, DIM)           # [h, p, n]
        woPf = np.ascontiguousarray(
            np.transpose(wo4, (1, 0, 2)).astype(BF_NP).reshape(128, QH * DIM))
        in_maps.append({
            "xP": xP,
            "wqP": pack_w(wq[:, i * QS:(i + 1) * QS], 512),
            "wkP": pack_w(wk[:, i * 128:(i + 1) * 128], 128),
            "wvP": pack_w(wv[:, i * 128:(i + 1) * 128], 128),
            "woP": woPf,
            "cosP": cosT,
            "sinP": sinTs,
            "stairP": stair,
            "onesP": ones,
        })
    return in_maps


def kernel(x, cos, sin, mask, wq, wk, wv, wo, _trace=False, _trace_kwargs=None):
    nc = _get_nc()
    in_maps = _host_prep(x, cos, sin, mask, wq, wk, wv, wo)
    res = run_bass_kernel_spmd(
        nc, in_maps, list(range(N_CORES)), trace=_trace,
        **(_trace_kwargs or {}),
    )
    partials = [np.asarray(res.results[i]["out"], dtype=np.float32)
                for i in range(N_CORES)]
    full = np.sum(np.stack(partials, axis=0), axis=0, dtype=np.float64)
    out = full.astype(np.float32)[None, :, :]
    if _trace:
        return out, res
    return out
